# revision 23
# baseline (speedup 1.0000x reference)
"""Self-contained Trainium2 Bass kernel for single-head T2T attention (fp8).

Problem: x:[8,4096,512], w_qkv:[1536,512], w_proj:[512,512], b_proj:[512]
    qkv = x @ w_qkv.T ; q,k,v split
    attn = softmax(q @ k.T / sqrt(512))
    out  = v + (attn @ v) @ w_proj.T + b_proj

Sharding: data-parallel over batch B=8 across the 8 NeuronCores (one
example per core); weights replicated.  No collectives needed.

Strategy: the output is v + o where |o|/|v| ~ 0.7% for this input
distribution, so the attention path tolerates fp8 easily while v (the
residual) is kept at fp32r accuracy.  All big matmuls except the V
projection run as float8e4 with MatmulPerfMode.DoubleRow: each instruction
contracts TWO 128-row k-tiles ([K,2,M] lhsT / [K,2,N] rhs).  Measured on
this hardware: 246.5 ns per DR matmul (K=256,M=128,N=512) vs 291.5 ns for
bf16/f32r (K=128) -- the weight load serializes with compute (walrus runs
with ldw-opt disabled), so per-instruction cost ~ K_load + N_compute and
the only lever is fewer/fuller PE instructions.

vs the first-generation kernel (503 us), three structural changes:
  1. G-fusion: S = Q.K^T = x (Wq^T Wk) x^T.  G = Wq^T Wk is computed
     on-device once (16 f32r matmuls, no weight transposes needed since
     both operands contract over the natural w_qkv row dim), cast to
     fp8(x128).  Phase 1 then computes (xG)^T (8 DR matmuls/chunk)
     instead of Q^T and K^T (16/chunk), and phase-2 scores use the
     already-resident x^T fp8 as the stationary side: S^T = x.(xG)^T.
     Saves ~16 us of PE plus 32 PE weight transposes.
  2. Softmax denominators as row matmuls: one ones-vector DR matmul per
     m-pair accumulating sum_m P into a [1,512] PSUM row (16/chunk),
     instead of 64/chunk per-q-block column matmuls whose cost is
     weight-load-bound.  The row is copied to SBUF (DVE) and moved into
     per-partition column form with 4 tiny K=1 matmuls against a 1x1
     identity slice.  Saves ~32 us of PE.
  3. PE transposes run as f32r (1.5 cycles/row vs 2.0 for f32) via
     bitcast views: x^T is consumed by an f32r matmul anyway, so the
     reduced PE mantissa changes nothing downstream.

Scale folding (no extra instructions, keeps every fp8 operand in the
normal range, |x| < 448):
    G8      = fp8(128 * G)                     (G std ~9e-3 -> ~1.2)
    gT      = G8.xT = 128*(xG)^T               (std ~26)
    scores  Sh = xT8.gT8 = 128*S               -> exp scale = SCALE/128
    exp     Ph = exp(Sh*scale + ln 64) = 64*P    (range ~[24, 180])
    v8      = fp8(V)                             (std ~0.45)
    ot      = sum_m Ph*v8 = 64*sum(P*V)          (std ~1900 in PSUM fp32)
    oT8     = fp8(ot / 64) = fp8(sum P*V)        (std ~29, max ~150)
    wproj8  = fp8(16 * w_proj)                -> pj = 16*(sum(P*V) @ Wp)
    ones    = 0.25                            -> sums = 16*sum(P)
    fin     = pj / sums + vres  ==  (P@V@Wp)/sum(P) + v   (exact folding)

Per-core dataflow (N=4096, C=512, P=128):
  phase 0: G = Wq^T Wk via 16 f32r matmuls on natural w_qkv row blocks
      (fp8 x128 copies); PE-transpose wv rows into wvr [c,C] f32r and
      w_proj into wproj8 [d,C] fp8 (x16 scale on the PSUM->SBUF copies).
  phase 1 (per 512-wide n-chunk): stream x, PE-transpose to x^T (f32r),
      copy to xTr f32r (ACT) and into the resident xT8 fp8 (Pool,
      SBUF->SBUF); V = x@wv in f32r with fp8 copy (ACT) and fp32
      residual copies (DVE/ACT; +bias on DVE when b_proj != 0);
      (xG)^T via fp8 DoubleRow against G8, fp8 copies into resident
      gT8 (DVE), one chunk behind the x^T cast chain.
  phase 2 (per 512-wide query chunk): m-loop over 16 m-block PAIRS:
      S^T pair-block via 4 DoubleRow matmuls into a [128,2,512] PSUM tile
      (st 2x2 banks + ot 4 banks = all 8 PSUM banks), ONE exp activation
      per pair ([128,1024] free, scores bounded so softmax without
      max-subtraction is safe), PV via 4 DoubleRow matmuls accumulating
      O^T in 4 PSUM banks.  The m-loop is software-pipelined (PV one pair
      behind exp).  Denominators: 16 ones-row DR matmuls AFTER the m-loop
      (pT_all stays chunk-resident) -> [1,512] row -> DVE copy -> 4 K=1
      matmuls -> [128,4] columns -> DVE reciprocal; the normalization is
      folded into the final scalar_tensor_tensor (it commutes with the
      row-wise linear proj).
"""

import numpy as np

import concourse.bass as bass
import concourse.mybir as mybir
from concourse.tile import TileContext
from concourse.masks import make_identity

P = 128
B = 8
N_FULL = 4096
C = 512
F = 3 * C
NQ = 512           # query chunk width (free dim of most matmuls)
CB = C // P        # 4 contraction sub-blocks of the model dim
SCALE = 1.0 / float(np.sqrt(C))
F32 = mybir.dt.float32
F32R = mybir.dt.float32r
FP8 = mybir.dt.float8e4
DR = mybir.MatmulPerfMode.DoubleRow

WS = 16.0          # weight pre-scale for the w_proj fp8 cast
SG = 128.0         # pre-scale for the G = Wq^T Wk fp8 cast
ES = 64.0          # exp output scale, applied via bias = ln(ES)
OS = 1.0 / 64.0    # scale on the O^T psum->fp8 copy (keeps |sum P*V| < fp8 max)
ONEV = ES * OS * WS / ES   # denominator const so recip folds exactly: 0.25


# ---------------------------------------------------------------------------
# Two sync post-passes.
#
# 1. Race fix: legalization splits each matmul into InstLdweights +
#    InstMatmult, but Tile's waits stay on the matmul.  The in-order PE
#    executes the ldweights FIRST, so a stationary operand produced by
#    another engine (oT8/gT8/G8 from DVE, v8/pT from ACT, xT8 from Pool)
#    can be read BEFORE the wait that guards it is enforced -- a
#    nondeterministic data race on hardware that CoreSim cannot see (it
#    does not model ldweights).  Hoist every matmul's waits onto its
#    immediately-preceding ldweights: semaphores are monotonic, so
#    waiting earlier is strictly safe.
#
# 2. Wait-cap legalization: this container's walrus build accepts at most
#    one sync wait per plain instruction (two for EventSemaphore), but
#    Tile's wait assignment can attach several.  Move excess waits onto
#    injected same-engine NOPs placed immediately before the
#    over-subscribed instruction.
# ---------------------------------------------------------------------------
def _legalize_waits(nc):
    for fn in nc.m.functions:
        for bb in fn.blocks:
            insts = bb.instructions
            prev = None
            for inst in insts:
                if (isinstance(inst, mybir.InstMatmult)
                        and isinstance(prev, mybir.InstLdweights)
                        and prev.engine == inst.engine):
                    mw = list(inst.sync_info.on_wait) if (
                        inst.sync_info and inst.sync_info.on_wait) else []
                    if mw:
                        lsi = prev.sync_info
                        lw = list(lsi.on_wait) if (
                            lsi and lsi.on_wait) else []
                        lup = list(lsi.on_update) if (
                            lsi and lsi.on_update) else []
                        prev.sync_info = mybir.SyncInfo(
                            on_wait=lw + mw, on_update=lup)
                        inst.sync_info.on_wait = []
                prev = inst
    for fn in nc.m.functions:
        for bb in fn.blocks:
            insts = bb.instructions
            out = []
            changed = False
            for inst in insts:
                si = inst.sync_info
                waits = list(si.on_wait) if si and si.on_wait else []
                cap = 2 if isinstance(inst, mybir.InstEventSemaphore) else 1
                if len(waits) > cap:
                    keep = waits[:cap]
                    rest = waits[cap:]
                    for i, w in enumerate(rest):
                        nop = mybir.InstNoOp(
                            name=f"{inst.name}-wspill{i}",
                            ins=[], outs=[], engine=inst.engine)
                        nop.sync_info = mybir.SyncInfo(
                            on_wait=[w], on_update=[])
                        nc.register_instruction(nop, overwrite=True)
                        out.append(nop)
                    si.on_wait = keep
                    changed = True
                out.append(inst)
            if changed:
                insts.clear()
                insts.extend(out)


class _nullctx:
    def __enter__(self):
        return None

    def __exit__(self, *a):
        return False


def build_program(n=N_FULL, reps=1, hw_loop=0, has_bias=False, variant="all"):
    """Build the per-core Bass program for one [n, C] example."""
    n_chunks = n // NQ
    mb_total = n // P
    npair = mb_total // 2

    nc = bass.Bass("TRN2", target_bir_lowering=False,
                   dynamic_dma_scratch_size=8192)
    x = nc.dram_tensor("x", (n, C), F32, kind="ExternalInput")
    w_qkv = nc.dram_tensor("w_qkv", (F, C), F32, kind="ExternalInput")
    w_proj = nc.dram_tensor("w_proj", (C, C), F32, kind="ExternalInput")
    b_proj = nc.dram_tensor("b_proj", (C,), F32, kind="ExternalInput")
    out = nc.dram_tensor("out", (n, C), F32, kind="ExternalOutput")

    def f32view(ap):
        # fp32r storage is fp32 bits; view as fp32 for non-PE ops
        return ap.bitcast(F32) if ap.dtype == F32R else ap


    with TileContext(nc) as tc:
        with tc.tile_pool(name="singles", bufs=1) as singles:
            ident = singles.tile([P, P], F32)
            make_identity(nc, ident)
            ones16 = singles.tile([P, 2, P], FP8)
            nc.vector.memset(ones16, ONEV)
            oneb = singles.tile([1, 1], mybir.dt.bfloat16)
            nc.vector.memset(oneb, 1.0)
            expbias = singles.tile([P, 1], F32)
            nc.vector.memset(expbias, float(np.log(ES)))
            bias_bc = singles.tile([P, C], F32)
            nc.sync.dma_start(
                out=bias_bc, in_=b_proj[:].unsqueeze(0).to_broadcast((P, C)))

            xT8 = singles.tile([P, CB, n], FP8)      # x^T: [c, n] fp8
            gT8 = singles.tile([P, CB, n], FP8)      # (xG)^T: [c, n] fp8 (x128)
            v8 = singles.tile([P, mb_total, C], FP8)   # V: [m, d] fp8
            vres = singles.tile([P, mb_total, C], F32)  # V + bias, exact
            G8 = singles.tile([P, CB, C], FP8)         # G: [cq, ck] fp8 (x128)
            wvr = singles.tile([P, CB, C], F32R)       # [c, d] f32r
            wproj8 = singles.tile([P, CB, C], FP8)     # [d, e] fp8 (x16)

            rep_ctx = (tc.For_i(0, hw_loop, 1) if hw_loop
                       else _nullctx())
            with rep_ctx:
              for _rep in range(reps):
                # ---- phase 0 + 1: G, weight transposes, x^T, V, (xG)^T ----
                with tc.tile_pool(name="wload", bufs=6) as wload, \
                     tc.tile_pool(name="xtr", bufs=3) as xtr_pool, \
                     tc.tile_pool(name="tp_psum", bufs=3, space="PSUM") as tp_psum, \
                     tc.tile_pool(name="qk_psum", bufs=2, space="PSUM") as qk_psum, \
                     tc.tile_pool(name="v_psum", bufs=2, space="PSUM") as v_psum:

                    def emit_xchunk(ch):
                        n0 = ch * NQ
                        xTr = xtr_pool.tile([P, CB, NQ], F32R, tag="xtr",
                                            name=f"xTr{ch}")
                        for nb in range(NQ // P):
                            xn = wload.tile([P, C], F32, tag="xn")
                            nc.sync.dma_start(
                                out=xn, in_=x[n0 + nb * P:n0 + (nb + 1) * P, :])
                            tp = tp_psum.tile([P, C], F32, tag="tp")
                            for cb in range(CB):
                                nc.tensor.transpose(
                                    tp[:, cb * P:(cb + 1) * P],
                                    xn[:, cb * P:(cb + 1) * P], ident)
                            nc.scalar.copy(
                                out=xTr[:, :, nb * P:(nb + 1) * P], in_=tp)
                            nc.gpsimd.tensor_copy(
                                out=xT8[:, :, n0 + nb * P:n0 + (nb + 1) * P],
                                in_=f32view(xTr[:, :, nb * P:(nb + 1) * P]))
                        # V (f32r, accuracy-critical residual); nb-pairs are
                        # interleaved across the cb chain so consecutive
                        # matmuls never hit the same PSUM region
                        for nb0 in range(0, NQ // P, 2):
                            vps = [v_psum.tile([P, NQ], F32, tag="v",
                                               name=f"vp{i}")
                                   for i in range(2)]
                            for cb in range(CB):
                                for i in range(2):
                                    nc.tensor.matmul(
                                        vps[i],
                                        xTr[:, cb, (nb0 + i) * P:(nb0 + i + 1) * P],
                                        wvr[:, cb, :],
                                        start=(cb == 0), stop=(cb == CB - 1))
                            for i in range(2):
                              nb = nb0 + i
                              vp = vps[i]
                              nc.scalar.copy(out=v8[:, ch * (NQ // P) + nb, :], in_=vp)
                            # Pool cannot touch PSUM.  ACT cannot apply a
                            # per-column bias, so with a bias all residual
                            # adds go to DVE; the common b_proj==0 case
                            # splits plain copies between DVE and ACT.
                              vdst = vres[:, ch * (NQ // P) + nb, :]
                              if has_bias:
                                  nc.vector.tensor_add(out=vdst, in0=vp, in1=bias_bc)
                              elif nb % 2 == 0:
                                  nc.vector.tensor_copy(out=vdst, in_=vp)
                              else:
                                  nc.scalar.copy(out=vdst, in_=vp)

                    def emit_xg(ch):
                        # (xG)^T chunk = G8-stationary DR matmuls over
                        # the resident x^T fp8 (single-bank tiles so the
                        # chunk-0 score run-ahead fits in PSUM)
                        n0 = ch * NQ
                        for ck in range(CB):
                            gp = qk_psum.tile([P, NQ], F32, tag="qk",
                                              name=f"gp{ck}")
                            for ci in range(2):
                                nc.tensor.matmul(
                                    gp,
                                    G8[:, 2 * ci:2 * ci + 2, ck * P:(ck + 1) * P],
                                    xT8[:, 2 * ci:2 * ci + 2, n0:n0 + NQ],
                                    start=(ci == 0), stop=(ci == 1),
                                    perf_mode=DR)
                            nc.vector.tensor_copy(
                                out=gT8[:, ck, n0:n0 + NQ], in_=gp)

                    # Emission order keeps the PE fed from the first DMA:
                    # wv transposes (short DMA lead-in), then x chunk 0,
                    # then the G build + w_proj transposes (their DMAs sit
                    # behind chunk 0's in the queue), then chunks 1..7 with
                    # the (xG)^T chain one chunk behind.
                    for rb in range(CB):              # 4 w_qkv V row blocks
                        wnat = wload.tile([P, C], F32, tag="wnat")
                        nc.sync.dma_start(
                            out=wnat, in_=w_qkv[2 * C + rb * P:2 * C + (rb + 1) * P, :])
                        tpw = tp_psum.tile([P, C], F32, tag="tp")
                        for cb in range(CB):
                            nc.tensor.transpose(
                                tpw[:, cb * P:(cb + 1) * P],
                                wnat[:, cb * P:(cb + 1) * P], ident)
                        nc.scalar.copy(
                            out=wvr[:, :, rb * P:(rb + 1) * P], in_=tpw)

                    emit_xchunk(0)

                    # G = Wq^T @ Wk: contraction over the 512 q-rows /
                    # 512 k-rows of w_qkv -- both operands are natural row
                    # blocks, no transposes (bf16 casts on Pool).  cq
                    # accumulators interleaved in pairs so consecutive
                    # matmuls never hit the same PSUM region.
                    with tc.tile_pool(name="gw", bufs=1) as gw:
                        BF16 = mybir.dt.bfloat16
                        wqn = [gw.tile([P, C], F32, tag=f"wq{fb}",
                                       name=f"wqn{fb}")
                               for fb in range(CB)]
                        wkn = [gw.tile([P, C], F32, tag=f"wk{fb}",
                                       name=f"wkn{fb}")
                               for fb in range(CB)]
                        wqb = [gw.tile([P, C], BF16, tag=f"wqb{fb}",
                                       name=f"wqb{fb}")
                               for fb in range(CB)]
                        wkb = [gw.tile([P, C], BF16, tag=f"wkb{fb}",
                                       name=f"wkb{fb}")
                               for fb in range(CB)]
                        for fb in range(CB):
                            nc.sync.dma_start(
                                out=wqn[fb], in_=w_qkv[fb * P:(fb + 1) * P, :])
                            nc.sync.dma_start(
                                out=wkn[fb],
                                in_=w_qkv[C + fb * P:C + (fb + 1) * P, :])
                            nc.gpsimd.tensor_copy(out=wqb[fb], in_=wqn[fb])
                            nc.gpsimd.tensor_copy(out=wkb[fb], in_=wkn[fb])
                        for cq0 in range(0, CB, 2):
                            gps = [tp_psum.tile([P, C], F32, tag="tp",
                                                name=f"gp{cq0 + i}")
                                   for i in range(2)]
                            for fb in range(CB):
                                for i in range(2):
                                    cq = cq0 + i
                                    nc.tensor.matmul(
                                        gps[i],
                                        wqb[fb][:, cq * P:(cq + 1) * P],
                                        wkb[fb],
                                        start=(fb == 0), stop=(fb == CB - 1))
                            for i in range(2):
                                nc.vector.tensor_scalar_mul(
                                    out=G8[:, cq0 + i, :], in0=gps[i],
                                    scalar1=SG)

                    for eb in range(C // P):          # 4 w_proj row blocks
                        wnat = wload.tile([P, C], F32, tag="wnat")
                        nc.sync.dma_start(out=wnat, in_=w_proj[eb * P:(eb + 1) * P, :])
                        tpw = tp_psum.tile([P, C], F32, tag="tp")
                        for db in range(CB):
                            nc.tensor.transpose(
                                tpw[:, db * P:(db + 1) * P],
                                wnat[:, db * P:(db + 1) * P], ident)
                        nc.vector.tensor_scalar_mul(
                            out=wproj8[:, :, eb * P:(eb + 1) * P],
                            in0=tpw, scalar1=WS)

                    for ch in range(1, n_chunks):
                        emit_xchunk(ch)
                        emit_xg(ch - 1)
                    emit_xg(n_chunks - 1)

                # ---- phase 2: attention + proj + residual ----
                if variant == "p1":
                    continue
                with tc.tile_pool(name="pT", bufs=2) as pT_pool, \
                     tc.tile_pool(name="oT8", bufs=2) as oT8_pool, \
                     tc.tile_pool(name="fin", bufs=3) as fin_pool, \
                     tc.tile_pool(name="rs", bufs=2) as rs_pool, \
                     tc.tile_pool(name="st_psum", bufs=2, space="PSUM") as st_psum, \
                     tc.tile_pool(name="ot_psum", bufs=4, space="PSUM") as ot_psum:

                    for ch in range(n_chunks):
                        n0 = ch * NQ
                        pT_all = pT_pool.tile([P, mb_total, NQ], FP8,
                                              tag="pT")
                        ot = [ot_psum.tile([P, NQ], F32, tag="ot", name=f"ot{db}")
                              for db in range(CB)]

                        def emit_pv(j):
                            for db in range(CB):
                                nc.tensor.matmul(
                                    ot[db],
                                    v8[:, 2 * j:2 * j + 2, db * P:(db + 1) * P],
                                    pT_all[:, 2 * j:2 * j + 2, :],
                                    start=(j == 0), stop=(j == npair - 1),
                                    perf_mode=DR)

                        # software-pipelined m-pair loop: PV runs TWO
                        # pairs behind the scores.  One-behind stalls the
                        # PE ~200ns per pair: the exp of a [128,2,512]
                        # pair (~1.13us on ACT) is slower than a 4-matmul
                        # scores block (~0.99us), so PV(j-1) would wait on
                        # exp(j-1) at every pair.  Two-behind gives the
                        # exp a full extra pair period of slack.
                        for j in range(npair):
                            st = st_psum.tile([P, 2, NQ], F32, tag="st")
                            for h in range(2):
                                mb = 2 * j + h
                                for ci in range(2):
                                    nc.tensor.matmul(
                                        st[:, h, :],
                                        xT8[:, 2 * ci:2 * ci + 2, mb * P:(mb + 1) * P],
                                        gT8[:, 2 * ci:2 * ci + 2, n0:n0 + NQ],
                                        start=(ci == 0), stop=(ci == 1),
                                        perf_mode=DR)
                            nc.scalar.activation(
                                out=pT_all[:, 2 * j:2 * j + 2, :], in_=st,
                                func=mybir.ActivationFunctionType.Exp,
                                scale=SCALE / SG,
                                bias=expbias)
                            if j >= 2:
                                emit_pv(j - 2)
                        emit_pv(npair - 2)
                        emit_pv(npair - 1)

                        # denominators: ones-row DR matmuls accumulate
                        # sum_m P as a [1,512] PSUM row (PE), overlapped
                        # with the oT8 copies (DVE); then the proj matmuls
                        # (into the freed ot banks) overlap with the DVE
                        # row copy + 4 tiny K=1 column-transpose matmuls.
                        sums_row = None
                        if variant != "nosums":
                            sums_row = st_psum.tile([P, NQ], F32, tag="st",
                                                    name=f"srow{ch}")
                            for j in range(npair):
                                nc.tensor.matmul(
                                    sums_row, ones16,
                                    pT_all[:, 2 * j:2 * j + 2, :],
                                    start=(j == 0), stop=(j == npair - 1),
                                    perf_mode=DR)

                        oT8 = oT8_pool.tile([P, CB, NQ], FP8, tag="oT8")
                        for db in range(CB):
                            nc.vector.tensor_scalar_mul(
                                out=oT8[:, db, :], in0=ot[db], scalar1=OS)

                        def emit_pj(nb):
                            pj = ot_psum.tile([P, C], F32, tag="ot",
                                              name=f"pj{nb}")
                            for ci in range(2):
                                nc.tensor.matmul(
                                    pj,
                                    oT8[:, 2 * ci:2 * ci + 2, nb * P:(nb + 1) * P],
                                    wproj8[:, 2 * ci:2 * ci + 2, :],
                                    start=(ci == 0), stop=(ci == 1),
                                    perf_mode=DR)
                            return pj

                        def emit_fin(nb, pj, recip):
                            fin = fin_pool.tile([P, C], F32, tag="fin")
                            # fin = pj * (1/rowsum) + (v + bias)
                            nc.vector.scalar_tensor_tensor(
                                out=fin, in0=pj,
                                scalar=recip[:, nb:nb + 1],
                                in1=vres[:, ch * (NQ // P) + nb, :],
                                op0=mybir.AluOpType.mult,
                                op1=mybir.AluOpType.add)
                            nc.sync.dma_start(
                                out=out[n0 + nb * P:n0 + (nb + 1) * P, :],
                                in_=fin)

                        # proj interleaved with the denominator column
                        # transpose so the fins (which recycle the ot/pj
                        # banks for the next chunk's PV) issue as early as
                        # possible
                        pj0 = emit_pj(0)
                        pj1 = emit_pj(1)
                        recip = rs_pool.tile([P, NQ // P], F32, tag="recip")
                        if variant == "nosums":
                            nc.vector.memset(recip, 2.4e-4)
                        else:
                            row_sb = rs_pool.tile([1, NQ], mybir.dt.bfloat16,
                                                  tag="rowsb",
                                                  name=f"rowsb{ch}")
                            nc.vector.tensor_copy(out=row_sb,
                                                  in_=sums_row[0:1, :])
                            tpr = st_psum.tile([P, NQ // P], F32, tag="st",
                                               name=f"tpr{ch}")
                            for nb in range(NQ // P):
                                nc.tensor.matmul(
                                    tpr[:, nb:nb + 1],
                                    row_sb[:, nb * P:(nb + 1) * P],
                                    oneb)
                            nc.vector.reciprocal(out=recip, in_=tpr)
                        emit_fin(0, pj0, recip)
                        emit_fin(1, pj1, recip)
                        pj2 = emit_pj(2)
                        pj3 = emit_pj(3)
                        emit_fin(2, pj2, recip)
                        emit_fin(3, pj3, recip)
    _legalize_waits(nc)
    return nc


_PROGRAM_CACHE = {}


def _get_program(n=N_FULL, reps=1, has_bias=False):
    key = (n, reps, has_bias)
    if key not in _PROGRAM_CACHE:
        _PROGRAM_CACHE[key] = build_program(n, reps=reps, has_bias=has_bias)
    return _PROGRAM_CACHE[key]


def kernel(x, w_qkv, w_proj, b_proj):
    from concourse.bass_utils import run_bass_kernel_spmd

    x = np.ascontiguousarray(np.asarray(x, dtype=np.float32))
    w_qkv = np.ascontiguousarray(np.asarray(w_qkv, dtype=np.float32))
    w_proj = np.ascontiguousarray(np.asarray(w_proj, dtype=np.float32))
    b_proj = np.ascontiguousarray(np.asarray(b_proj, dtype=np.float32))
    b, n, c = x.shape
    assert (b, n, c) == (B, N_FULL, C)

    nc = _get_program(has_bias=bool(np.any(b_proj != 0.0)))
    in_maps = [
        {"x": x[i], "w_qkv": w_qkv, "w_proj": w_proj, "b_proj": b_proj}
        for i in range(B)
    ]
    res = run_bass_kernel_spmd(nc, in_maps, list(range(B)))
    return np.stack([res.results[i]["out"] for i in range(B)], axis=0)


# revision 24
# speedup vs baseline: 1.1311x; 1.1311x over previous
"""Self-contained Trainium2 Bass kernel for single-head T2T attention (fp8).

Problem: x:[8,4096,512], w_qkv:[1536,512], w_proj:[512,512], b_proj:[512]
    qkv = x @ w_qkv.T ; q,k,v split
    attn = softmax(q @ k.T / sqrt(512))
    out  = v + (attn @ v) @ w_proj.T + b_proj

Sharding: data-parallel over batch B=8 across the 8 NeuronCores (one
example per core); weights replicated.  No collectives needed.

Strategy: the output is v + o where |o|/|v| ~ 0.7% for this input
distribution, so the attention path tolerates fp8 easily while v (the
residual) is kept at fp32r accuracy.  All big matmuls except the V
projection run as float8e4 with MatmulPerfMode.DoubleRow: each instruction
contracts TWO 128-row k-tiles ([K,2,M] lhsT / [K,2,N] rhs).  Measured on
this hardware: 246.5 ns per DR matmul (K=256,M=128,N=512) vs 291.5 ns for
bf16/f32r (K=128) -- the weight load serializes with compute (walrus runs
with ldw-opt disabled), so per-instruction cost ~ K_load + N_compute and
the only lever is fewer/fuller PE instructions.

vs the first-generation kernel (503 us), three structural changes:
  1. G-fusion: S = Q.K^T = x (Wq^T Wk) x^T.  G = Wq^T Wk is computed
     on-device once (16 f32r matmuls, no weight transposes needed since
     both operands contract over the natural w_qkv row dim), cast to
     fp8(x128).  Phase 1 then computes (xG)^T (8 DR matmuls/chunk)
     instead of Q^T and K^T (16/chunk), and phase-2 scores use the
     already-resident x^T fp8 as the stationary side: S^T = x.(xG)^T.
     Saves ~16 us of PE plus 32 PE weight transposes.
  2. Softmax denominators as row matmuls: one ones-vector DR matmul per
     m-pair accumulating sum_m P into a [1,512] PSUM row (16/chunk),
     instead of 64/chunk per-q-block column matmuls whose cost is
     weight-load-bound.  The row is copied to SBUF (DVE) and moved into
     per-partition column form with 4 tiny K=1 matmuls against a 1x1
     identity slice.  Saves ~32 us of PE.
  3. PE transposes run as f32r (1.5 cycles/row vs 2.0 for f32) via
     bitcast views: x^T is consumed by an f32r matmul anyway, so the
     reduced PE mantissa changes nothing downstream.

Scale folding (no extra instructions, keeps every fp8 operand in the
normal range, |x| < 448):
    G8      = fp8(128 * G)                     (G std ~9e-3 -> ~1.2)
    gT      = G8.xT = 128*(xG)^T               (std ~26)
    scores  Sh = xT8.gT8 = 128*S               -> exp scale = SCALE/128
    exp     Ph = exp(Sh*scale + ln 64) = 64*P    (range ~[24, 180])
    v8      = fp8(V)                             (std ~0.45)
    ot      = sum_m Ph*v8 = 64*sum(P*V)          (std ~1900 in PSUM fp32)
    oT8     = fp8(ot / 64) = fp8(sum P*V)        (std ~29, max ~150)
    wproj8  = fp8(16 * w_proj)                -> pj = 16*(sum(P*V) @ Wp)
    ones    = 0.25                            -> sums = 16*sum(P)
    fin     = pj / sums + vres  ==  (P@V@Wp)/sum(P) + v   (exact folding)

Per-core dataflow (N=4096, C=512, P=128):
  phase 0: G = Wq^T Wk via 16 f32r matmuls on natural w_qkv row blocks
      (fp8 x128 copies); PE-transpose wv rows into wvr [c,C] f32r and
      w_proj into wproj8 [d,C] fp8 (x16 scale on the PSUM->SBUF copies).
  phase 1 (per 512-wide n-chunk): stream x, PE-transpose to x^T (f32r),
      copy to xTr f32r (ACT) and into the resident xT8 fp8 (Pool,
      SBUF->SBUF); V = x@wv in f32r with fp8 copy (ACT) and fp32
      residual copies (DVE/ACT; +bias on DVE when b_proj != 0);
      (xG)^T via fp8 DoubleRow against G8, fp8 copies into resident
      gT8 (DVE), one chunk behind the x^T cast chain.
  phase 2 (per 512-wide query chunk): m-loop over 16 m-block PAIRS:
      S^T pair-block via 4 DoubleRow matmuls into a [128,2,512] PSUM tile
      (st 2x2 banks + ot 4 banks = all 8 PSUM banks), ONE exp activation
      per pair ([128,1024] free, scores bounded so softmax without
      max-subtraction is safe), PV via 4 DoubleRow matmuls accumulating
      O^T in 4 PSUM banks.  The m-loop is software-pipelined (PV one pair
      behind exp).  Denominators: 16 ones-row DR matmuls AFTER the m-loop
      (pT_all stays chunk-resident) -> [1,512] row -> DVE copy -> 4 K=1
      matmuls -> [128,4] columns -> DVE reciprocal; the normalization is
      folded into the final scalar_tensor_tensor (it commutes with the
      row-wise linear proj).
"""

import numpy as np

import concourse.bass as bass
import concourse.mybir as mybir
from concourse.tile import TileContext
from concourse.masks import make_identity

P = 128
B = 8
N_FULL = 4096
C = 512
F = 3 * C
NQ = 512           # query chunk width (free dim of most matmuls)
CB = C // P        # 4 contraction sub-blocks of the model dim
SCALE = 1.0 / float(np.sqrt(C))
F32 = mybir.dt.float32
F32R = mybir.dt.float32r
FP8 = mybir.dt.float8e4
DR = mybir.MatmulPerfMode.DoubleRow

WS = 16.0          # weight pre-scale for the w_proj fp8 cast
SG = 128.0         # pre-scale for the G = Wq^T Wk fp8 cast
ES = 64.0          # exp output scale, applied via bias = ln(ES)
OS = 1.0 / 64.0    # scale on the O^T psum->fp8 copy (keeps |sum P*V| < fp8 max)
ONEV = ES * OS * WS / ES   # denominator const so recip folds exactly: 0.25


# ---------------------------------------------------------------------------
# Two sync post-passes.
#
# 1. Race fix: legalization splits each matmul into InstLdweights +
#    InstMatmult, but Tile's waits stay on the matmul.  The in-order PE
#    executes the ldweights FIRST, so a stationary operand produced by
#    another engine (oT8/gT8/G8 from DVE, v8/pT from ACT, xT8 from Pool)
#    can be read BEFORE the wait that guards it is enforced -- a
#    nondeterministic data race on hardware that CoreSim cannot see (it
#    does not model ldweights).  Hoist every matmul's waits onto its
#    immediately-preceding ldweights: semaphores are monotonic, so
#    waiting earlier is strictly safe.
#
# 2. Wait-cap legalization: this container's walrus build accepts at most
#    one sync wait per plain instruction (two for EventSemaphore), but
#    Tile's wait assignment can attach several.  Move excess waits onto
#    injected same-engine NOPs placed immediately before the
#    over-subscribed instruction.
# ---------------------------------------------------------------------------
def _legalize_waits(nc):
    for fn in nc.m.functions:
        for bb in fn.blocks:
            insts = bb.instructions
            prev = None
            for inst in insts:
                if (isinstance(inst, mybir.InstMatmult)
                        and isinstance(prev, mybir.InstLdweights)
                        and prev.engine == inst.engine):
                    mw = list(inst.sync_info.on_wait) if (
                        inst.sync_info and inst.sync_info.on_wait) else []
                    if mw:
                        lsi = prev.sync_info
                        lw = list(lsi.on_wait) if (
                            lsi and lsi.on_wait) else []
                        lup = list(lsi.on_update) if (
                            lsi and lsi.on_update) else []
                        prev.sync_info = mybir.SyncInfo(
                            on_wait=lw + mw, on_update=lup)
                        inst.sync_info.on_wait = []
                prev = inst
    for fn in nc.m.functions:
        for bb in fn.blocks:
            insts = bb.instructions
            out = []
            changed = False
            for inst in insts:
                si = inst.sync_info
                waits = list(si.on_wait) if si and si.on_wait else []
                cap = 2 if isinstance(inst, mybir.InstEventSemaphore) else 1
                if len(waits) > cap:
                    keep = waits[:cap]
                    rest = waits[cap:]
                    for i, w in enumerate(rest):
                        nop = mybir.InstNoOp(
                            name=f"{inst.name}-wspill{i}",
                            ins=[], outs=[], engine=inst.engine)
                        nop.sync_info = mybir.SyncInfo(
                            on_wait=[w], on_update=[])
                        nc.register_instruction(nop, overwrite=True)
                        out.append(nop)
                    si.on_wait = keep
                    changed = True
                out.append(inst)
            if changed:
                insts.clear()
                insts.extend(out)


class _nullctx:
    def __enter__(self):
        return None

    def __exit__(self, *a):
        return False


def build_program(n=N_FULL, reps=1, hw_loop=0, has_bias=False, variant="all",
                  pv_lag=2):
    """Build the per-core Bass program for one [n, C] example."""
    n_chunks = n // NQ
    mb_total = n // P
    npair = mb_total // 2

    nc = bass.Bass("TRN2", target_bir_lowering=False,
                   dynamic_dma_scratch_size=8192)
    x = nc.dram_tensor("x", (n, C), F32, kind="ExternalInput")
    w_qkv = nc.dram_tensor("w_qkv", (F, C), F32, kind="ExternalInput")
    w_proj = nc.dram_tensor("w_proj", (C, C), F32, kind="ExternalInput")
    b_proj = nc.dram_tensor("b_proj", (C,), F32, kind="ExternalInput")
    out = nc.dram_tensor("out", (n, C), F32, kind="ExternalOutput")

    def f32view(ap):
        # fp32r storage is fp32 bits; view as fp32 for non-PE ops
        return ap.bitcast(F32) if ap.dtype == F32R else ap


    with TileContext(nc) as tc:
        with tc.tile_pool(name="singles", bufs=1) as singles:
            ident = singles.tile([P, P], F32)
            make_identity(nc, ident)
            ones16 = singles.tile([P, 2, P], FP8)
            nc.vector.memset(ones16, ONEV)
            oneb = singles.tile([1, 1], mybir.dt.bfloat16)
            nc.vector.memset(oneb, 1.0)
            expbias = singles.tile([P, 1], F32)
            nc.vector.memset(expbias, float(np.log(ES)))
            bias_bc = singles.tile([P, C], F32)
            nc.sync.dma_start(
                out=bias_bc, in_=b_proj[:].unsqueeze(0).to_broadcast((P, C)))

            xT8 = singles.tile([P, CB, n], FP8)      # x^T: [c, n] fp8
            gT8 = singles.tile([P, CB, n], FP8)      # (xG)^T: [c, n] fp8 (x128)
            v8 = singles.tile([P, mb_total, C], FP8)   # V: [m, d] fp8
            vres = singles.tile([P, mb_total, C], F32)  # V + bias, exact
            G8 = singles.tile([P, CB, C], FP8)         # G: [cq, ck] fp8 (x128)
            wvr = singles.tile([P, CB, C], F32R)       # [c, d] f32r
            wproj8 = singles.tile([P, CB, C], FP8)     # [d, e] fp8 (x16)

            rep_ctx = (tc.For_i(0, hw_loop, 1) if hw_loop
                       else _nullctx())
            with rep_ctx:
              for _rep in range(reps):
                # ---- phase 0 + 1: G, weight transposes, x^T, V, (xG)^T ----
                with tc.tile_pool(name="wload", bufs=6) as wload, \
                     tc.tile_pool(name="xtr", bufs=3) as xtr_pool, \
                     tc.tile_pool(name="tp_psum", bufs=3, space="PSUM") as tp_psum, \
                     tc.tile_pool(name="qk_psum", bufs=2, space="PSUM") as qk_psum, \
                     tc.tile_pool(name="v_psum", bufs=2, space="PSUM") as v_psum:

                    def emit_xchunk(ch):
                        n0 = ch * NQ
                        xTr = xtr_pool.tile([P, CB, NQ], F32R, tag="xtr",
                                            name=f"xTr{ch}")
                        for nb in range(NQ // P):
                            xn = wload.tile([P, C], F32, tag="xn")
                            nc.sync.dma_start(
                                out=xn, in_=x[n0 + nb * P:n0 + (nb + 1) * P, :])
                            tp = tp_psum.tile([P, C], F32, tag="tp")
                            for cb in range(CB):
                                nc.tensor.transpose(
                                    tp[:, cb * P:(cb + 1) * P],
                                    xn[:, cb * P:(cb + 1) * P], ident)
                            nc.scalar.copy(
                                out=xTr[:, :, nb * P:(nb + 1) * P], in_=tp)
                            nc.gpsimd.tensor_copy(
                                out=xT8[:, :, n0 + nb * P:n0 + (nb + 1) * P],
                                in_=f32view(xTr[:, :, nb * P:(nb + 1) * P]))
                        # V (f32r, accuracy-critical residual); nb-pairs are
                        # interleaved across the cb chain so consecutive
                        # matmuls never hit the same PSUM region
                        for nb0 in range(0, NQ // P, 2):
                            vps = [v_psum.tile([P, NQ], F32, tag="v",
                                               name=f"vp{i}")
                                   for i in range(2)]
                            for cb in range(CB):
                                for i in range(2):
                                    nc.tensor.matmul(
                                        vps[i],
                                        xTr[:, cb, (nb0 + i) * P:(nb0 + i + 1) * P],
                                        wvr[:, cb, :],
                                        start=(cb == 0), stop=(cb == CB - 1))
                            for i in range(2):
                              nb = nb0 + i
                              vp = vps[i]
                              nc.scalar.copy(out=v8[:, ch * (NQ // P) + nb, :], in_=vp)
                            # Pool cannot touch PSUM.  ACT cannot apply a
                            # per-column bias, so with a bias all residual
                            # adds go to DVE; the common b_proj==0 case
                            # splits plain copies between DVE and ACT.
                              vdst = vres[:, ch * (NQ // P) + nb, :]
                              if has_bias:
                                  nc.vector.tensor_add(out=vdst, in0=vp, in1=bias_bc)
                              elif nb % 2 == 0:
                                  nc.vector.tensor_copy(out=vdst, in_=vp)
                              else:
                                  nc.scalar.copy(out=vdst, in_=vp)

                    def emit_xg(ch):
                        # (xG)^T chunk = G8-stationary DR matmuls over
                        # the resident x^T fp8 (single-bank tiles so the
                        # chunk-0 score run-ahead fits in PSUM)
                        n0 = ch * NQ
                        for ck in range(CB):
                            gp = qk_psum.tile([P, NQ], F32, tag="qk",
                                              name=f"gp{ck}")
                            for ci in range(2):
                                nc.tensor.matmul(
                                    gp,
                                    G8[:, 2 * ci:2 * ci + 2, ck * P:(ck + 1) * P],
                                    xT8[:, 2 * ci:2 * ci + 2, n0:n0 + NQ],
                                    start=(ci == 0), stop=(ci == 1),
                                    perf_mode=DR)
                            nc.vector.tensor_copy(
                                out=gT8[:, ck, n0:n0 + NQ], in_=gp)

                    # Emission order keeps the PE fed from the first DMA:
                    # wv transposes (short DMA lead-in), then x chunk 0,
                    # then the G build + w_proj transposes (their DMAs sit
                    # behind chunk 0's in the queue), then chunks 1..7 with
                    # the (xG)^T chain one chunk behind.
                    for rb in range(CB):              # 4 w_qkv V row blocks
                        wnat = wload.tile([P, C], F32, tag="wnat")
                        nc.sync.dma_start(
                            out=wnat, in_=w_qkv[2 * C + rb * P:2 * C + (rb + 1) * P, :])
                        tpw = tp_psum.tile([P, C], F32, tag="tp")
                        for cb in range(CB):
                            nc.tensor.transpose(
                                tpw[:, cb * P:(cb + 1) * P],
                                wnat[:, cb * P:(cb + 1) * P], ident)
                        nc.scalar.copy(
                            out=wvr[:, :, rb * P:(rb + 1) * P], in_=tpw)

                    emit_xchunk(0)

                    # G = Wq^T @ Wk: contraction over the 512 q-rows /
                    # 512 k-rows of w_qkv -- both operands are natural row
                    # blocks, no transposes (bf16 casts on Pool).  cq
                    # accumulators interleaved in pairs so consecutive
                    # matmuls never hit the same PSUM region.
                    with tc.tile_pool(name="gw", bufs=1) as gw:
                        BF16 = mybir.dt.bfloat16
                        wqn = [gw.tile([P, C], F32, tag=f"wq{fb}",
                                       name=f"wqn{fb}")
                               for fb in range(CB)]
                        wkn = [gw.tile([P, C], F32, tag=f"wk{fb}",
                                       name=f"wkn{fb}")
                               for fb in range(CB)]
                        wqb = [gw.tile([P, C], BF16, tag=f"wqb{fb}",
                                       name=f"wqb{fb}")
                               for fb in range(CB)]
                        wkb = [gw.tile([P, C], BF16, tag=f"wkb{fb}",
                                       name=f"wkb{fb}")
                               for fb in range(CB)]
                        for fb in range(CB):
                            nc.sync.dma_start(
                                out=wqn[fb], in_=w_qkv[fb * P:(fb + 1) * P, :])
                            nc.sync.dma_start(
                                out=wkn[fb],
                                in_=w_qkv[C + fb * P:C + (fb + 1) * P, :])
                            nc.gpsimd.tensor_copy(out=wqb[fb], in_=wqn[fb])
                            nc.gpsimd.tensor_copy(out=wkb[fb], in_=wkn[fb])
                        for cq0 in range(0, CB, 2):
                            gps = [tp_psum.tile([P, C], F32, tag="tp",
                                                name=f"gp{cq0 + i}")
                                   for i in range(2)]
                            for fb in range(CB):
                                for i in range(2):
                                    cq = cq0 + i
                                    nc.tensor.matmul(
                                        gps[i],
                                        wqb[fb][:, cq * P:(cq + 1) * P],
                                        wkb[fb],
                                        start=(fb == 0), stop=(fb == CB - 1))
                            for i in range(2):
                                nc.vector.tensor_scalar_mul(
                                    out=G8[:, cq0 + i, :], in0=gps[i],
                                    scalar1=SG)

                    for eb in range(C // P):          # 4 w_proj row blocks
                        wnat = wload.tile([P, C], F32, tag="wnat")
                        nc.sync.dma_start(out=wnat, in_=w_proj[eb * P:(eb + 1) * P, :])
                        tpw = tp_psum.tile([P, C], F32, tag="tp")
                        for db in range(CB):
                            nc.tensor.transpose(
                                tpw[:, db * P:(db + 1) * P],
                                wnat[:, db * P:(db + 1) * P], ident)
                        nc.vector.tensor_scalar_mul(
                            out=wproj8[:, :, eb * P:(eb + 1) * P],
                            in0=tpw, scalar1=WS)

                    for ch in range(1, n_chunks):
                        emit_xchunk(ch)
                        emit_xg(ch - 1)
                    emit_xg(n_chunks - 1)

                # ---- phase 2: attention + proj + residual ----
                if variant == "p1":
                    continue
                with tc.tile_pool(name="pT", bufs=2) as pT_pool, \
                     tc.tile_pool(name="oT8", bufs=2) as oT8_pool, \
                     tc.tile_pool(name="fin", bufs=3) as fin_pool, \
                     tc.tile_pool(name="rs", bufs=2) as rs_pool, \
                     tc.tile_pool(name="st_psum", bufs=2, space="PSUM") as st_psum, \
                     tc.tile_pool(name="ot_psum", bufs=4, space="PSUM") as ot_psum:

                    for ch in range(n_chunks):
                        n0 = ch * NQ
                        pT_all = pT_pool.tile([P, mb_total, NQ], FP8,
                                              tag="pT")
                        ot = [ot_psum.tile([P, NQ], F32, tag="ot", name=f"ot{db}")
                              for db in range(CB)]

                        def emit_pv(j):
                            for db in range(CB):
                                nc.tensor.matmul(
                                    ot[db],
                                    v8[:, 2 * j:2 * j + 2, db * P:(db + 1) * P],
                                    pT_all[:, 2 * j:2 * j + 2, :],
                                    start=(j == 0), stop=(j == npair - 1),
                                    perf_mode=DR)

                        # software-pipelined m-pair loop: PV runs TWO
                        # pairs behind the scores.  One-behind stalls the
                        # PE ~200ns per pair: the exp of a [128,2,512]
                        # pair (~1.13us on ACT) is slower than a 4-matmul
                        # scores block (~0.99us), so PV(j-1) would wait on
                        # exp(j-1) at every pair.  Two-behind gives the
                        # exp a full extra pair period of slack.
                        for j in range(npair):
                            st = st_psum.tile([P, 2, NQ], F32, tag="st")
                            for h in range(2):
                                mb = 2 * j + h
                                for ci in range(2):
                                    nc.tensor.matmul(
                                        st[:, h, :],
                                        xT8[:, 2 * ci:2 * ci + 2, mb * P:(mb + 1) * P],
                                        gT8[:, 2 * ci:2 * ci + 2, n0:n0 + NQ],
                                        start=(ci == 0), stop=(ci == 1),
                                        perf_mode=DR)
                            nc.scalar.activation(
                                out=pT_all[:, 2 * j:2 * j + 2, :], in_=st,
                                func=mybir.ActivationFunctionType.Exp,
                                scale=SCALE / SG,
                                bias=expbias)
                            if j >= pv_lag:
                                emit_pv(j - pv_lag)
                        for j in range(npair - pv_lag, npair):
                            emit_pv(j)

                        # denominators: ones-row DR matmuls accumulate
                        # sum_m P as a [1,512] PSUM row (PE), overlapped
                        # with the oT8 copies (DVE); then the proj matmuls
                        # (into the freed ot banks) overlap with the DVE
                        # row copy + 4 tiny K=1 column-transpose matmuls.
                        sums_row = None
                        if variant != "nosums":
                            sums_row = st_psum.tile([P, NQ], F32, tag="st",
                                                    name=f"srow{ch}")
                            for j in range(npair):
                                nc.tensor.matmul(
                                    sums_row, ones16,
                                    pT_all[:, 2 * j:2 * j + 2, :],
                                    start=(j == 0), stop=(j == npair - 1),
                                    perf_mode=DR)

                        oT8 = oT8_pool.tile([P, CB, NQ], FP8, tag="oT8")
                        for db in range(CB):
                            nc.vector.tensor_scalar_mul(
                                out=oT8[:, db, :], in0=ot[db], scalar1=OS)

                        def emit_pj(nb):
                            pj = ot_psum.tile([P, C], F32, tag="ot",
                                              name=f"pj{nb}")
                            for ci in range(2):
                                nc.tensor.matmul(
                                    pj,
                                    oT8[:, 2 * ci:2 * ci + 2, nb * P:(nb + 1) * P],
                                    wproj8[:, 2 * ci:2 * ci + 2, :],
                                    start=(ci == 0), stop=(ci == 1),
                                    perf_mode=DR)
                            return pj

                        def emit_fin(nb, pj, recip):
                            fin = fin_pool.tile([P, C], F32, tag="fin")
                            # fin = pj * (1/rowsum) + (v + bias)
                            nc.vector.scalar_tensor_tensor(
                                out=fin, in0=pj,
                                scalar=recip[:, nb:nb + 1],
                                in1=vres[:, ch * (NQ // P) + nb, :],
                                op0=mybir.AluOpType.mult,
                                op1=mybir.AluOpType.add)
                            nc.sync.dma_start(
                                out=out[n0 + nb * P:n0 + (nb + 1) * P, :],
                                in_=fin)

                        # proj interleaved with the denominator column
                        # transpose so the fins (which recycle the ot/pj
                        # banks for the next chunk's PV) issue as early as
                        # possible
                        pj0 = emit_pj(0)
                        pj1 = emit_pj(1)
                        recip = rs_pool.tile([P, NQ // P], F32, tag="recip")
                        if variant == "nosums":
                            nc.vector.memset(recip, 2.4e-4)
                        else:
                            row_sb = rs_pool.tile([1, NQ], mybir.dt.bfloat16,
                                                  tag="rowsb",
                                                  name=f"rowsb{ch}")
                            nc.vector.tensor_copy(out=row_sb,
                                                  in_=sums_row[0:1, :])
                            tpr = st_psum.tile([P, NQ // P], F32, tag="st",
                                               name=f"tpr{ch}")
                            for nb in range(NQ // P):
                                nc.tensor.matmul(
                                    tpr[:, nb:nb + 1],
                                    row_sb[:, nb * P:(nb + 1) * P],
                                    oneb)
                            nc.vector.reciprocal(out=recip, in_=tpr)
                        emit_fin(0, pj0, recip)
                        emit_fin(1, pj1, recip)
                        pj2 = emit_pj(2)
                        pj3 = emit_pj(3)
                        emit_fin(2, pj2, recip)
                        emit_fin(3, pj3, recip)
    _legalize_waits(nc)
    return nc


_PROGRAM_CACHE = {}


def _get_program(n=N_FULL, reps=1, has_bias=False):
    key = (n, reps, has_bias)
    if key not in _PROGRAM_CACHE:
        _PROGRAM_CACHE[key] = build_program(n, reps=reps, has_bias=has_bias)
    return _PROGRAM_CACHE[key]


def kernel(x, w_qkv, w_proj, b_proj):
    from concourse.bass_utils import run_bass_kernel_spmd

    x = np.ascontiguousarray(np.asarray(x, dtype=np.float32))
    w_qkv = np.ascontiguousarray(np.asarray(w_qkv, dtype=np.float32))
    w_proj = np.ascontiguousarray(np.asarray(w_proj, dtype=np.float32))
    b_proj = np.ascontiguousarray(np.asarray(b_proj, dtype=np.float32))
    b, n, c = x.shape
    assert (b, n, c) == (B, N_FULL, C)

    nc = _get_program(has_bias=bool(np.any(b_proj != 0.0)))
    in_maps = [
        {"x": x[i], "w_qkv": w_qkv, "w_proj": w_proj, "b_proj": b_proj}
        for i in range(B)
    ]
    res = run_bass_kernel_spmd(nc, in_maps, list(range(B)))
    return np.stack([res.results[i]["out"] for i in range(B)], axis=0)


# revision 27
# speedup vs baseline: 1.1753x; 1.0392x over previous
"""Self-contained Trainium2 Bass kernel for single-head T2T attention (fp8).

Problem: x:[8,4096,512], w_qkv:[1536,512], w_proj:[512,512], b_proj:[512]
    qkv = x @ w_qkv.T ; q,k,v split
    attn = softmax(q @ k.T / sqrt(512))
    out  = v + (attn @ v) @ w_proj.T + b_proj

Sharding: data-parallel over batch B=8 across the 8 NeuronCores (one
example per core); weights replicated.  No collectives needed.

Strategy: the output is v + o where |o|/|v| ~ 0.7% for this input
distribution, so the attention path tolerates fp8 easily while v (the
residual) is kept at fp32r accuracy.  All big matmuls except the V
projection run as float8e4 with MatmulPerfMode.DoubleRow: each instruction
contracts TWO 128-row k-tiles ([K,2,M] lhsT / [K,2,N] rhs).  Measured on
this hardware: 246.5 ns per DR matmul (K=256,M=128,N=512) vs 291.5 ns for
bf16/f32r (K=128) -- the weight load serializes with compute (walrus runs
with ldw-opt disabled), so per-instruction cost ~ K_load + N_compute and
the only lever is fewer/fuller PE instructions.

vs the first-generation kernel (503 us -> ~436 us), the changes:
  1. G-fusion: S = Q.K^T = x (Wq^T Wk) x^T.  G = Wq^T Wk is computed
     on-device once (16 bf16 matmuls on NATURAL w_qkv row blocks -- the
     contraction is over the qkv row dim, so no weight transposes --
     then fp8(x128) casts).  Phase 1 computes (xG)^T (8 DR matmuls per
     512-chunk) instead of Q^T and K^T (16/chunk), and phase-2 scores
     reuse the already-resident x^T fp8 as the stationary side:
     S^T = x.(xG)^T.  Same exp scale as the old Qx16/Kx16 path.
  2. Softmax denominators as broadcast-row matmuls: one all-ones
     [K,2,128] DR stationary per m-pair accumulates sum_m P into a
     [128,512] PSUM tile (every partition identical; 16 matmuls/chunk
     at full-matmul cost) instead of 64/chunk per-q-block column
     matmuls (weight-load-bound; small-M DR ldweights also fails the
     walrus ISA check).  Row 0 is copied to SBUF bf16 (DVE) and moved
     into per-partition column form with 4 tiny K=1 bf16 matmuls
     against a 1x1 one; DVE reciprocal; the normalization folds into
     the final scalar_tensor_tensor (commutes with the row-linear
     proj).
  3. Ldweights data-race fix: legalization splits each matmul into
     InstLdweights + InstMatmult but Tile leaves the waits on the
     matmul, so the in-order PE can load a stationary operand BEFORE
     the producing engine's write lands (CoreSim does not model
     ldweights and cannot see it; on HW it corrupts o
     nondeterministically).  _legalize_waits hoists every matmul's
     waits onto its paired ldweights -- semaphores are monotonic, so
     waiting earlier is strictly safe.
  4. Emission-order and pipeline-depth tuning (A/B-measured on HW with
     interleaved slope benches): wv transposes -> x chunk 0 -> G ->
     w_proj -> chunks 1..7, so the PE never idles behind the weight
     DMAs; proj matmuls interleaved with the denominator transpose so
     the fins (which recycle the ot/pj banks for the next chunk's PV)
     issue early; wload/xtr/tp_psum deepened to 6/3/3 (a ~29 us win);
     PV one pair behind exp and v_psum=2 (deeper was measured slower).

Scale folding (no extra instructions, keeps every fp8 operand in the
normal range, |x| < 448):
    G8      = fp8(128 * G)                     (G std ~9e-3 -> ~1.2)
    gT      = G8.xT = 128*(xG)^T               (std ~26)
    scores  Sh = xT8.gT8 = 128*S               -> exp scale = SCALE/128
    exp     Ph = exp(Sh*scale + ln 64) = 64*P    (range ~[24, 180])
    v8      = fp8(V)                             (std ~0.45)
    ot      = sum_m Ph*v8 = 64*sum(P*V)          (std ~1900 in PSUM fp32)
    oT8     = fp8(ot / 64) = fp8(sum P*V)        (std ~29, max ~150)
    wproj8  = fp8(16 * w_proj)                -> pj = 16*(sum(P*V) @ Wp)
    ones    = 0.25                            -> sums = 16*sum(P)
    fin     = pj / sums + vres  ==  (P@V@Wp)/sum(P) + v   (exact folding)

Per-core dataflow (N=4096, C=512, P=128):
  phase 0/1 (per 512-wide n-chunk): stream x, PE-transpose to x^T (f32),
      copy to xTr f32r (ACT) and into the resident xT8 fp8 (Pool,
      SBUF->SBUF); V = x@wv in f32r with fp8 copy (ACT) and fp32
      residual copies (DVE/ACT; +bias on DVE when b_proj != 0);
      (xG)^T via fp8 DoubleRow against G8 into resident gT8 (DVE
      copies), one chunk behind the x^T cast chain.
  phase 2 (per 512-wide query chunk): m-loop over 16 m-block PAIRS:
      S^T pair-block via 4 DoubleRow matmuls into a [128,2,512] PSUM
      tile (st 2x2 banks + ot 4 banks = all 8 PSUM banks), ONE exp
      activation per pair ([128,1024] free, scores bounded so softmax
      without max-subtraction is safe), PV via 4 DoubleRow matmuls
      accumulating O^T in 4 PSUM banks, one pair behind exp.
      Denominators + proj + residual per the folding above; everything
      stays in SBUF -- no DRAM scratch.
"""

import numpy as np

import concourse.bass as bass
import concourse.mybir as mybir
from concourse.tile import TileContext
from concourse.masks import make_identity

P = 128
B = 8
N_FULL = 4096
C = 512
F = 3 * C
NQ = 512           # query chunk width (free dim of most matmuls)
CB = C // P        # 4 contraction sub-blocks of the model dim
SCALE = 1.0 / float(np.sqrt(C))
F32 = mybir.dt.float32
F32R = mybir.dt.float32r
FP8 = mybir.dt.float8e4
DR = mybir.MatmulPerfMode.DoubleRow

WS = 16.0          # weight pre-scale for the w_proj fp8 cast
SG = 128.0         # pre-scale for the G = Wq^T Wk fp8 cast
ES = 64.0          # exp output scale, applied via bias = ln(ES)
OS = 1.0 / 64.0    # scale on the O^T psum->fp8 copy (keeps |sum P*V| < fp8 max)
ONEV = ES * OS * WS / ES   # denominator const so recip folds exactly: 0.25


# ---------------------------------------------------------------------------
# Two sync post-passes.
#
# 1. Race fix: legalization splits each matmul into InstLdweights +
#    InstMatmult, but Tile's waits stay on the matmul.  The in-order PE
#    executes the ldweights FIRST, so a stationary operand produced by
#    another engine (oT8/gT8/G8 from DVE, v8/pT from ACT, xT8 from Pool)
#    can be read BEFORE the wait that guards it is enforced -- a
#    nondeterministic data race on hardware that CoreSim cannot see (it
#    does not model ldweights).  Hoist every matmul's waits onto its
#    immediately-preceding ldweights: semaphores are monotonic, so
#    waiting earlier is strictly safe.
#
# 2. Wait-cap legalization: this container's walrus build accepts at most
#    one sync wait per plain instruction (two for EventSemaphore), but
#    Tile's wait assignment can attach several.  Move excess waits onto
#    injected same-engine NOPs placed immediately before the
#    over-subscribed instruction.
# ---------------------------------------------------------------------------
def _legalize_waits(nc):
    for fn in nc.m.functions:
        for bb in fn.blocks:
            insts = bb.instructions
            prev = None
            for inst in insts:
                if (isinstance(inst, mybir.InstMatmult)
                        and isinstance(prev, mybir.InstLdweights)
                        and prev.engine == inst.engine):
                    mw = list(inst.sync_info.on_wait) if (
                        inst.sync_info and inst.sync_info.on_wait) else []
                    if mw:
                        lsi = prev.sync_info
                        lw = list(lsi.on_wait) if (
                            lsi and lsi.on_wait) else []
                        lup = list(lsi.on_update) if (
                            lsi and lsi.on_update) else []
                        prev.sync_info = mybir.SyncInfo(
                            on_wait=lw + mw, on_update=lup)
                        inst.sync_info.on_wait = []
                prev = inst
    for fn in nc.m.functions:
        for bb in fn.blocks:
            insts = bb.instructions
            out = []
            changed = False
            for inst in insts:
                si = inst.sync_info
                waits = list(si.on_wait) if si and si.on_wait else []
                cap = 2 if isinstance(inst, mybir.InstEventSemaphore) else 1
                if len(waits) > cap:
                    keep = waits[:cap]
                    rest = waits[cap:]
                    for i, w in enumerate(rest):
                        nop = mybir.InstNoOp(
                            name=f"{inst.name}-wspill{i}",
                            ins=[], outs=[], engine=inst.engine)
                        nop.sync_info = mybir.SyncInfo(
                            on_wait=[w], on_update=[])
                        nc.register_instruction(nop, overwrite=True)
                        out.append(nop)
                    si.on_wait = keep
                    changed = True
                out.append(inst)
            if changed:
                insts.clear()
                insts.extend(out)


class _nullctx:
    def __enter__(self):
        return None

    def __exit__(self, *a):
        return False


def build_program(n=N_FULL, reps=1, hw_loop=0, has_bias=False, variant="all",
                  pv_lag=1, depths=(6, 3, 3), v_bufs=2):
    """Build the per-core Bass program for one [n, C] example."""
    n_chunks = n // NQ
    mb_total = n // P
    npair = mb_total // 2

    nc = bass.Bass("TRN2", target_bir_lowering=False,
                   dynamic_dma_scratch_size=8192)
    x = nc.dram_tensor("x", (n, C), F32, kind="ExternalInput")
    w_qkv = nc.dram_tensor("w_qkv", (F, C), F32, kind="ExternalInput")
    w_proj = nc.dram_tensor("w_proj", (C, C), F32, kind="ExternalInput")
    b_proj = nc.dram_tensor("b_proj", (C,), F32, kind="ExternalInput")
    out = nc.dram_tensor("out", (n, C), F32, kind="ExternalOutput")

    def f32view(ap):
        # fp32r storage is fp32 bits; view as fp32 for non-PE ops
        return ap.bitcast(F32) if ap.dtype == F32R else ap


    with TileContext(nc) as tc:
        with tc.tile_pool(name="singles", bufs=1) as singles:
            ident = singles.tile([P, P], F32)
            make_identity(nc, ident)
            ones16 = singles.tile([P, 2, P], FP8)
            nc.vector.memset(ones16, ONEV)
            oneb = singles.tile([1, 1], mybir.dt.bfloat16)
            nc.vector.memset(oneb, 1.0)
            expbias = singles.tile([P, 1], F32)
            nc.vector.memset(expbias, float(np.log(ES)))
            bias_bc = singles.tile([P, C], F32)
            nc.sync.dma_start(
                out=bias_bc, in_=b_proj[:].unsqueeze(0).to_broadcast((P, C)))

            xT8 = singles.tile([P, CB, n], FP8)      # x^T: [c, n] fp8
            gT8 = singles.tile([P, CB, n], FP8)      # (xG)^T: [c, n] fp8 (x128)
            v8 = singles.tile([P, mb_total, C], FP8)   # V: [m, d] fp8
            vres = singles.tile([P, mb_total, C], F32)  # V + bias, exact
            G8 = singles.tile([P, CB, C], FP8)         # G: [cq, ck] fp8 (x128)
            wvr = singles.tile([P, CB, C], F32R)       # [c, d] f32r
            wproj8 = singles.tile([P, CB, C], FP8)     # [d, e] fp8 (x16)

            rep_ctx = (tc.For_i(0, hw_loop, 1) if hw_loop
                       else _nullctx())
            with rep_ctx:
              for _rep in range(reps):
                # ---- phase 0 + 1: G, weight transposes, x^T, V, (xG)^T ----
                with tc.tile_pool(name="wload", bufs=depths[0]) as wload, \
                     tc.tile_pool(name="xtr", bufs=depths[1]) as xtr_pool, \
                     tc.tile_pool(name="tp_psum", bufs=depths[2], space="PSUM") as tp_psum, \
                     tc.tile_pool(name="qk_psum", bufs=2, space="PSUM") as qk_psum, \
                     tc.tile_pool(name="v_psum", bufs=v_bufs, space="PSUM") as v_psum:

                    def emit_xchunk(ch):
                        n0 = ch * NQ
                        xTr = xtr_pool.tile([P, CB, NQ], F32R, tag="xtr",
                                            name=f"xTr{ch}")
                        for nb in range(NQ // P):
                            xn = wload.tile([P, C], F32, tag="xn")
                            nc.sync.dma_start(
                                out=xn, in_=x[n0 + nb * P:n0 + (nb + 1) * P, :])
                            tp = tp_psum.tile([P, C], F32, tag="tp")
                            for cb in range(CB):
                                nc.tensor.transpose(
                                    tp[:, cb * P:(cb + 1) * P],
                                    xn[:, cb * P:(cb + 1) * P], ident)
                            nc.scalar.copy(
                                out=xTr[:, :, nb * P:(nb + 1) * P], in_=tp)
                            nc.gpsimd.tensor_copy(
                                out=xT8[:, :, n0 + nb * P:n0 + (nb + 1) * P],
                                in_=f32view(xTr[:, :, nb * P:(nb + 1) * P]))
                        # V (f32r, accuracy-critical residual); nb-pairs are
                        # interleaved across the cb chain so consecutive
                        # matmuls never hit the same PSUM region
                        for nb0 in range(0, NQ // P, 2):
                            vps = [v_psum.tile([P, NQ], F32, tag="v",
                                               name=f"vp{i}")
                                   for i in range(2)]
                            for cb in range(CB):
                                for i in range(2):
                                    nc.tensor.matmul(
                                        vps[i],
                                        xTr[:, cb, (nb0 + i) * P:(nb0 + i + 1) * P],
                                        wvr[:, cb, :],
                                        start=(cb == 0), stop=(cb == CB - 1))
                            for i in range(2):
                              nb = nb0 + i
                              vp = vps[i]
                              nc.scalar.copy(out=v8[:, ch * (NQ // P) + nb, :], in_=vp)
                            # Pool cannot touch PSUM.  ACT cannot apply a
                            # per-column bias, so with a bias all residual
                            # adds go to DVE; the common b_proj==0 case
                            # splits plain copies between DVE and ACT.
                              vdst = vres[:, ch * (NQ // P) + nb, :]
                              if has_bias:
                                  nc.vector.tensor_add(out=vdst, in0=vp, in1=bias_bc)
                              elif nb % 2 == 0:
                                  nc.vector.tensor_copy(out=vdst, in_=vp)
                              else:
                                  nc.scalar.copy(out=vdst, in_=vp)

                    def emit_xg(ch):
                        # (xG)^T chunk = G8-stationary DR matmuls over
                        # the resident x^T fp8 (single-bank tiles so the
                        # chunk-0 score run-ahead fits in PSUM)
                        n0 = ch * NQ
                        for ck in range(CB):
                            gp = qk_psum.tile([P, NQ], F32, tag="qk",
                                              name=f"gp{ck}")
                            for ci in range(2):
                                nc.tensor.matmul(
                                    gp,
                                    G8[:, 2 * ci:2 * ci + 2, ck * P:(ck + 1) * P],
                                    xT8[:, 2 * ci:2 * ci + 2, n0:n0 + NQ],
                                    start=(ci == 0), stop=(ci == 1),
                                    perf_mode=DR)
                            nc.vector.tensor_copy(
                                out=gT8[:, ck, n0:n0 + NQ], in_=gp)

                    # Emission order keeps the PE fed from the first DMA:
                    # wv transposes (short DMA lead-in), then x chunk 0,
                    # then the G build + w_proj transposes (their DMAs sit
                    # behind chunk 0's in the queue), then chunks 1..7 with
                    # the (xG)^T chain one chunk behind.
                    for rb in range(CB):              # 4 w_qkv V row blocks
                        wnat = wload.tile([P, C], F32, tag="wnat")
                        nc.sync.dma_start(
                            out=wnat, in_=w_qkv[2 * C + rb * P:2 * C + (rb + 1) * P, :])
                        tpw = tp_psum.tile([P, C], F32, tag="tp")
                        for cb in range(CB):
                            nc.tensor.transpose(
                                tpw[:, cb * P:(cb + 1) * P],
                                wnat[:, cb * P:(cb + 1) * P], ident)
                        nc.scalar.copy(
                            out=wvr[:, :, rb * P:(rb + 1) * P], in_=tpw)

                    emit_xchunk(0)

                    # G = Wq^T @ Wk: contraction over the 512 q-rows /
                    # 512 k-rows of w_qkv -- both operands are natural row
                    # blocks, no transposes (bf16 casts on Pool).  cq
                    # accumulators interleaved in pairs so consecutive
                    # matmuls never hit the same PSUM region.
                    with tc.tile_pool(name="gw", bufs=1) as gw:
                        BF16 = mybir.dt.bfloat16
                        wqn = [gw.tile([P, C], F32, tag=f"wq{fb}",
                                       name=f"wqn{fb}")
                               for fb in range(CB)]
                        wkn = [gw.tile([P, C], F32, tag=f"wk{fb}",
                                       name=f"wkn{fb}")
                               for fb in range(CB)]
                        wqb = [gw.tile([P, C], BF16, tag=f"wqb{fb}",
                                       name=f"wqb{fb}")
                               for fb in range(CB)]
                        wkb = [gw.tile([P, C], BF16, tag=f"wkb{fb}",
                                       name=f"wkb{fb}")
                               for fb in range(CB)]
                        for fb in range(CB):
                            nc.sync.dma_start(
                                out=wqn[fb], in_=w_qkv[fb * P:(fb + 1) * P, :])
                            nc.sync.dma_start(
                                out=wkn[fb],
                                in_=w_qkv[C + fb * P:C + (fb + 1) * P, :])
                            nc.gpsimd.tensor_copy(out=wqb[fb], in_=wqn[fb])
                            nc.gpsimd.tensor_copy(out=wkb[fb], in_=wkn[fb])
                        for cq0 in range(0, CB, 2):
                            gps = [tp_psum.tile([P, C], F32, tag="tp",
                                                name=f"gp{cq0 + i}")
                                   for i in range(2)]
                            for fb in range(CB):
                                for i in range(2):
                                    cq = cq0 + i
                                    nc.tensor.matmul(
                                        gps[i],
                                        wqb[fb][:, cq * P:(cq + 1) * P],
                                        wkb[fb],
                                        start=(fb == 0), stop=(fb == CB - 1))
                            for i in range(2):
                                nc.vector.tensor_scalar_mul(
                                    out=G8[:, cq0 + i, :], in0=gps[i],
                                    scalar1=SG)

                    for eb in range(C // P):          # 4 w_proj row blocks
                        wnat = wload.tile([P, C], F32, tag="wnat")
                        nc.sync.dma_start(out=wnat, in_=w_proj[eb * P:(eb + 1) * P, :])
                        tpw = tp_psum.tile([P, C], F32, tag="tp")
                        for db in range(CB):
                            nc.tensor.transpose(
                                tpw[:, db * P:(db + 1) * P],
                                wnat[:, db * P:(db + 1) * P], ident)
                        nc.vector.tensor_scalar_mul(
                            out=wproj8[:, :, eb * P:(eb + 1) * P],
                            in0=tpw, scalar1=WS)

                    for ch in range(1, n_chunks):
                        emit_xchunk(ch)
                        emit_xg(ch - 1)
                    emit_xg(n_chunks - 1)

                # ---- phase 2: attention + proj + residual ----
                if variant == "p1":
                    continue
                with tc.tile_pool(name="pT", bufs=2) as pT_pool, \
                     tc.tile_pool(name="oT8", bufs=2) as oT8_pool, \
                     tc.tile_pool(name="fin", bufs=3) as fin_pool, \
                     tc.tile_pool(name="rs", bufs=2) as rs_pool, \
                     tc.tile_pool(name="st_psum", bufs=2, space="PSUM") as st_psum, \
                     tc.tile_pool(name="ot_psum", bufs=4, space="PSUM") as ot_psum:

                    for ch in range(n_chunks):
                        n0 = ch * NQ
                        pT_all = pT_pool.tile([P, mb_total, NQ], FP8,
                                              tag="pT")
                        ot = [ot_psum.tile([P, NQ], F32, tag="ot", name=f"ot{db}")
                              for db in range(CB)]

                        def emit_pv(j):
                            for db in range(CB):
                                nc.tensor.matmul(
                                    ot[db],
                                    v8[:, 2 * j:2 * j + 2, db * P:(db + 1) * P],
                                    pT_all[:, 2 * j:2 * j + 2, :],
                                    start=(j == 0), stop=(j == npair - 1),
                                    perf_mode=DR)

                        # software-pipelined m-pair loop: PV runs TWO
                        # pairs behind the scores.  One-behind stalls the
                        # PE ~200ns per pair: the exp of a [128,2,512]
                        # pair (~1.13us on ACT) is slower than a 4-matmul
                        # scores block (~0.99us), so PV(j-1) would wait on
                        # exp(j-1) at every pair.  Two-behind gives the
                        # exp a full extra pair period of slack.
                        for j in range(npair):
                            st = st_psum.tile([P, 2, NQ], F32, tag="st")
                            for h in range(2):
                                mb = 2 * j + h
                                for ci in range(2):
                                    nc.tensor.matmul(
                                        st[:, h, :],
                                        xT8[:, 2 * ci:2 * ci + 2, mb * P:(mb + 1) * P],
                                        gT8[:, 2 * ci:2 * ci + 2, n0:n0 + NQ],
                                        start=(ci == 0), stop=(ci == 1),
                                        perf_mode=DR)
                            nc.scalar.activation(
                                out=pT_all[:, 2 * j:2 * j + 2, :], in_=st,
                                func=mybir.ActivationFunctionType.Exp,
                                scale=SCALE / SG,
                                bias=expbias)
                            if j >= pv_lag:
                                emit_pv(j - pv_lag)
                        for j in range(npair - pv_lag, npair):
                            emit_pv(j)

                        # denominators: ones-row DR matmuls accumulate
                        # sum_m P as a [1,512] PSUM row (PE), overlapped
                        # with the oT8 copies (DVE); then the proj matmuls
                        # (into the freed ot banks) overlap with the DVE
                        # row copy + 4 tiny K=1 column-transpose matmuls.
                        sums_row = None
                        if variant != "nosums":
                            sums_row = st_psum.tile([P, NQ], F32, tag="st",
                                                    name=f"srow{ch}")
                            for j in range(npair):
                                nc.tensor.matmul(
                                    sums_row, ones16,
                                    pT_all[:, 2 * j:2 * j + 2, :],
                                    start=(j == 0), stop=(j == npair - 1),
                                    perf_mode=DR)

                        oT8 = oT8_pool.tile([P, CB, NQ], FP8, tag="oT8")
                        for db in range(CB):
                            nc.vector.tensor_scalar_mul(
                                out=oT8[:, db, :], in0=ot[db], scalar1=OS)

                        def emit_pj(nb):
                            pj = ot_psum.tile([P, C], F32, tag="ot",
                                              name=f"pj{nb}")
                            for ci in range(2):
                                nc.tensor.matmul(
                                    pj,
                                    oT8[:, 2 * ci:2 * ci + 2, nb * P:(nb + 1) * P],
                                    wproj8[:, 2 * ci:2 * ci + 2, :],
                                    start=(ci == 0), stop=(ci == 1),
                                    perf_mode=DR)
                            return pj

                        def emit_fin(nb, pj, recip):
                            fin = fin_pool.tile([P, C], F32, tag="fin")
                            # fin = pj * (1/rowsum) + (v + bias)
                            nc.vector.scalar_tensor_tensor(
                                out=fin, in0=pj,
                                scalar=recip[:, nb:nb + 1],
                                in1=vres[:, ch * (NQ // P) + nb, :],
                                op0=mybir.AluOpType.mult,
                                op1=mybir.AluOpType.add)
                            nc.sync.dma_start(
                                out=out[n0 + nb * P:n0 + (nb + 1) * P, :],
                                in_=fin)

                        # proj interleaved with the denominator column
                        # transpose so the fins (which recycle the ot/pj
                        # banks for the next chunk's PV) issue as early as
                        # possible
                        pj0 = emit_pj(0)
                        pj1 = emit_pj(1)
                        recip = rs_pool.tile([P, NQ // P], F32, tag="recip")
                        if variant == "nosums":
                            nc.vector.memset(recip, 2.4e-4)
                        else:
                            row_sb = rs_pool.tile([1, NQ], mybir.dt.bfloat16,
                                                  tag="rowsb",
                                                  name=f"rowsb{ch}")
                            nc.vector.tensor_copy(out=row_sb,
                                                  in_=sums_row[0:1, :])
                            tpr = st_psum.tile([P, NQ // P], F32, tag="st",
                                               name=f"tpr{ch}")
                            for nb in range(NQ // P):
                                nc.tensor.matmul(
                                    tpr[:, nb:nb + 1],
                                    row_sb[:, nb * P:(nb + 1) * P],
                                    oneb)
                            nc.vector.reciprocal(out=recip, in_=tpr)
                        emit_fin(0, pj0, recip)
                        emit_fin(1, pj1, recip)
                        pj2 = emit_pj(2)
                        pj3 = emit_pj(3)
                        emit_fin(2, pj2, recip)
                        emit_fin(3, pj3, recip)
    _legalize_waits(nc)
    return nc


_PROGRAM_CACHE = {}


def _get_program(n=N_FULL, reps=1, has_bias=False):
    key = (n, reps, has_bias)
    if key not in _PROGRAM_CACHE:
        _PROGRAM_CACHE[key] = build_program(n, reps=reps, has_bias=has_bias)
    return _PROGRAM_CACHE[key]


def kernel(x, w_qkv, w_proj, b_proj):
    from concourse.bass_utils import run_bass_kernel_spmd

    x = np.ascontiguousarray(np.asarray(x, dtype=np.float32))
    w_qkv = np.ascontiguousarray(np.asarray(w_qkv, dtype=np.float32))
    w_proj = np.ascontiguousarray(np.asarray(w_proj, dtype=np.float32))
    b_proj = np.ascontiguousarray(np.asarray(b_proj, dtype=np.float32))
    b, n, c = x.shape
    assert (b, n, c) == (B, N_FULL, C)

    nc = _get_program(has_bias=bool(np.any(b_proj != 0.0)))
    in_maps = [
        {"x": x[i], "w_qkv": w_qkv, "w_proj": w_proj, "b_proj": b_proj}
        for i in range(B)
    ]
    res = run_bass_kernel_spmd(nc, in_maps, list(range(B)))
    return np.stack([res.results[i]["out"] for i in range(B)], axis=0)
